# revision 48
# baseline (speedup 1.0000x reference)
"""Trainium2 Bass kernel for nn_ActorCritic (value MLP + per-sample hypernetwork).

Sharding: pure data parallel. Batch 4096 split as 512 samples per core across
8 NeuronCores; the small value-network weights are replicated (host
pre-transposed/packed so the device never transposes anything).

Per-core work:
  - Value net (TensorE, bf16): X^T [256,512] -> W1^T matmuls -> ELU -> W2^T
    -> ELU -> w3 -> value [512]. Kept in [feature, batch] layout so biases are
    per-partition ACT scalars. ELU(z) = relu(z) + exp(min(z,0)) - 1; the -1
    is folded into the *next* layer's bias on the host (b' = b - W.sum(axis=1)),
    and the relu/exp branches are kept as separate bf16 tiles -- the next
    layer's matmul runs over both and accumulates in PSUM (linearity), so no
    elementwise add is ever needed.
  - Hypernetwork (VectorE): option rows hold per-sample MLP weights
    (64x256, 64, 64x64, 64, 32x64, 32 packed), stored bf16. With samples on
    partitions, a custom DVE op computes a running dot product
    scan(ADD, Src0*Src1) over the weight stream against a stride-0-broadcast
    activation vector; segment sums are extracted by strided subtraction of
    the prefix sums (GpSimd does the small fix-up ops).
"""

import numpy as np

P = 128
B_LOCAL = 512
NBLK = B_LOCAL // P  # 4 blocks of 128 samples
NCORES = 8
Z = 256
H = 1024
OPT_W = 22688
OUT_W = 33

# hypernet packing offsets (HyperOption z=256->64->64->32)
OFF_W1, OFF_B1 = 0, 16384
OFF_W2, OFF_B2 = 16448, 20544
OFF_W3, OFF_B3 = 20608, 22656

# device-side repacked option stream: per layer, segment o = [w_o | b_o | 0],
# so a prefix-sum difference over one segment directly yields w_o . x + b_o.
# Segments are padded to even length so the DVE 2x packed-pair mode applies.
# L1: 64 segments of 258; L2: 64 of 66; L3: 32 of 66.
SEG1, SEG23 = 258, 66
RE_L2 = 64 * SEG1  # 16512
RE_L3 = RE_L2 + 64 * SEG23  # 20736
OPT_RE_W = RE_L3 + 32 * SEG23  # 22848


def _repack_option(opt):
    """[B, 22688] f32 -> [B, 22848] fp16 padded-segment layout."""
    B = opt.shape[0]
    out = np.zeros((B, OPT_RE_W), dtype=np.float16)
    l1 = out[:, :RE_L2].reshape(B, 64, SEG1)
    l1[:, :, 0:256] = opt[:, OFF_W1 : OFF_W1 + 16384].reshape(B, 64, 256)
    l1[:, :, 256] = opt[:, OFF_B1 : OFF_B1 + 64]
    l2 = out[:, RE_L2:RE_L3].reshape(B, 64, SEG23)
    l2[:, :, 0:64] = opt[:, OFF_W2 : OFF_W2 + 4096].reshape(B, 64, 64)
    l2[:, :, 64] = opt[:, OFF_B2 : OFF_B2 + 64]
    l3 = out[:, RE_L3:].reshape(B, 32, SEG23)
    l3[:, :, 0:64] = opt[:, OFF_W3 : OFF_W3 + 2048].reshape(B, 32, 64)
    l3[:, :, 64] = opt[:, OFF_B3 : OFF_B3 + 32]
    return out

# packed bf16 value-net matmul operands: xt | w1t | w2t | w3t
WPACK_W = 2 * 512 + 2 * 1024 + 8 * 1024 + 8  # 11272
# packed f32 biases: b1 | -b1 | b2' | -b2' | b3'
BPACK_W = 8 + 8 + 8 + 8 + 1  # 33

_NC_CACHE = {}
USE_2X = True  # set False to force the REGULAR (1x) scan program


def _bf16(a):
    import ml_dtypes

    return np.asarray(a, dtype=ml_dtypes.bfloat16)


def _register_op(name, make_spec):
    """Register (once) a custom DVE op by name; returns the DveOp."""
    from concourse.dve_spec import lower
    from concourse import dve_ops
    from concourse.dve_uop import DveOpSpec

    for op in dve_ops.OPS:
        if op.name == name:
            return op
    spec = make_spec()
    row = dve_ops._CUSTOM_DVE_ROW_BASE + len(dve_ops.OPS)
    assert row < 0x20
    dve_ops._SUB_OPCODE_FOR_NAME[name] = row
    shas = {}
    for ver in ("v3", "v4"):
        tmp = DveOpSpec(name=name, opcode=row, uops=lower(spec, ver=ver), rd1_en=True)
        shas[ver] = tmp.sha(ver)
    op = dve_ops.DveOp(name, spec, subdim=False, uops_sha=shas)
    dve_ops.OPS.append(op)
    dve_ops.CUSTOM_DVE_SPECS[name] = spec
    return op


def _register_mul_scan():
    """out[p,k] = sum_{j<=k} in0[p,j]*in1[p,j] (fp32 running dot product)."""
    from concourse.dve_spec import Spec, Src0, Src1, AluOp, scan

    def mk():
        def _ref(in0, in1, s0, s1, imm2):
            p = in0.shape[0]
            prod = in0.astype(np.float32).reshape(p, -1) * in1.astype(
                np.float32
            ).reshape(p, -1)
            return np.cumsum(prod, axis=-1).reshape(in0.shape).astype(np.float32)

        return Spec(body=scan(AluOp.ADD, Src0 * Src1), reference=_ref)

    return _register_op("MUL_SCAN_ANT", mk)


def _register_mul_scan2():
    """Running dot product with a hand-authored 2X_1PORT uop variant.

    REGULAR program comes from lower(); the 2x program processes two packed
    fp16 elements per cycle: m0=a0*b0, m1=a1*b1, q=m0+m1, s=q+s_prev
    (single-block feedback), lo=s-m1; writes (lo, s) to WR0_LO/WR0_HI.
    The engine falls back to REGULAR at runtime if the access pattern is
    ineligible, so correctness never depends on 2x engaging.
    """
    from concourse.dve_spec import Spec, Src0, Src1, AluOp as SAluOp, scan, lower
    from concourse import dve_ops
    from concourse.dve_uop import (
        DveOpSpec,
        UopConfig,
        AluOp as UAluOp,
        AluInp,
        DelayInp,
        InpSel,
        OutSel,
        OutPath,
        Trigger,
        ENABLE,
    )

    name = "MUL_SCAN2_ANT"
    for op in dve_ops.OPS:
        if op.name == name:
            return op

    def _ref(in0, in1, s0, s1, imm2):
        p = in0.shape[0]
        prod = in0.astype(np.float32).reshape(p, -1) * in1.astype(
            np.float32
        ).reshape(p, -1)
        return np.cumsum(prod, axis=-1).reshape(in0.shape).astype(np.float32)

    spec = Spec(body=scan(SAluOp.ADD, Src0 * Src1), reference=_ref)

    def mk_seed():
        u = UopConfig()
        u.enable_input(InpSel.ZERO, 5)  # delay chain 4 carries 0 to block 3
        b = u.datapath_config
        for k in (0, 1, 2):
            b[k].pass_through_alu()
            b[k].pass_through_delay(4)
        b[3].enable_alu(UAluOp.BYPASS, AluInp.PREV_DELAY_4, AluInp.PREV_DELAY_4)
        for k in (4, 5, 6, 7):
            b[k].pass_through_alu()
        u.repeat_count = 1
        u.trigger = (Trigger.COUNT, Trigger.NONE, Trigger.NONE)
        u.next_uop = (1, 0, 0)
        return u

    def mk_steady():
        u = UopConfig()
        u.enable_input(InpSel.SRC_0, 1)
        u.enable_input(InpSel.SRC_1, 2)
        u.enable_input(InpSel.SRC_0_HI, 3)
        u.enable_input(InpSel.SRC_1_HI, 4)
        b = u.datapath_config
        # m0 = a0*b0; carry (a1, b1) forward
        b[0].enable_alu(UAluOp.MULTIPLY, AluInp.PREV_DELAY_0, AluInp.PREV_DELAY_1)
        b[0].pass_through_delay(2, 3)
        # m1 = a1*b1; chain0 <- m0
        b[1].enable_alu(UAluOp.MULTIPLY, AluInp.PREV_DELAY_2, AluInp.PREV_DELAY_3)
        b[1].enable_delay_from_src(DelayInp.PREV_ALU_OUT, 0)
        # q = m1 + m0; chain1 <- m1
        b[2].enable_alu(UAluOp.ADD, AluInp.PREV_ALU_OUT, AluInp.PREV_DELAY_0)
        b[2].enable_delay_from_src(DelayInp.PREV_ALU_OUT, 1)
        # s = q + s_prev (feedback); carry m1
        b[3].enable_alu(UAluOp.ADD, AluInp.PREV_ALU_OUT, AluInp.CURR_ALU_OUT)
        b[3].pass_through_delay(1)
        # lo = s - m1; chain0 <- s
        b[4].enable_alu(UAluOp.SUBTRACT, AluInp.PREV_ALU_OUT, AluInp.PREV_DELAY_1)
        b[4].enable_delay_from_src(DelayInp.PREV_ALU_OUT, 0)
        for k in (5, 6, 7):
            b[k].pass_through_alu()
            b[k].pass_through_delay(0)
        u.enable_output(OutSel.ALU_OUT, OutPath.WR0_LO)
        u.enable_output(OutSel.DELAY_0, OutPath.WR0_HI)
        u.require_inp0 = ENABLE
        u.require_inp1 = ENABLE
        u.trigger = (Trigger.SRC_TENSOR_DONE, Trigger.NONE, Trigger.NONE)
        u.next_uop = (0, 0, 0)
        return u

    row = dve_ops._CUSTOM_DVE_ROW_BASE + len(dve_ops.OPS)
    assert row < 0x20
    dve_ops._SUB_OPCODE_FOR_NAME[name] = row

    class Scan2Op:
        def __init__(self):
            self.name = name
            self.spec = spec
            self.subdim = False
            self._cache = {}

        def compile(self, ver):
            if ver not in self._cache:
                kw = {}
                if ver == "v3":
                    kw = dict(uops_2x=[mk_seed(), mk_steady()], perf_max=1)
                self._cache[ver] = DveOpSpec(
                    name=name,
                    opcode=row,
                    uops=lower(spec, ver=ver),
                    rd1_en=True,
                    **kw,
                )
            return self._cache[ver]

    op = Scan2Op()
    dve_ops.OPS.append(op)
    dve_ops.CUSTOM_DVE_SPECS[name] = spec
    return op


def _emit_scan2(nc, op, out, in0, in1):
    """nc.vector._custom_dve equivalent with the 2x perf-mode bit set."""
    from concourse import bass_isa, mybir

    v = nc.vector
    from concourse.dve_table_gen import dve_ver_for

    op.compile(dve_ver_for(nc.trn_type))
    if op.name not in nc.m.ant_custom_dve_ops:
        nc.m.ant_custom_dve_ops = sorted({*nc.m.ant_custom_dve_ops, op.name})
    from concourse.dve_ops import get_dve_sub_opcode

    shape = (
        bass_isa.CustomDveShape.STT
        if len(in1.shape) > 2
        else bass_isa.CustomDveShape.TTSS
    )
    isa_opcode = nc.isa.Opcode[
        f"NEURON_ISA_TPB_OPCODE_CUSTOM_DVE_ANT_{shape.slot()}"
    ].value
    zero = mybir.ImmediateValue(dtype=mybir.dt.float32, value=0.0)
    return v.add_instruction(
        bass_isa.InstCustomDveAnt(
            name=nc.get_next_instruction_name(),
            op_name=op.name,
            rd1_en=True,
            subdim=0,
            imm2=0.0,
            shape=shape,
            row=get_dve_sub_opcode(op.name),
            isa_opcode=isa_opcode,
            ins=[
                v.lower_ap(in0, for_isa=True, opt=True),
                v.lower_ap(in1, for_isa=True, opt=True),
                zero,
                zero,
            ],
            outs=[v.lower_ap(out, for_isa=True, opt=True)],
            perf_max=1 if USE_2X else 0,
        )
    )


def _register_sub_relu():
    """out = relu(in0 - in1)."""
    from concourse.dve_spec import Spec, Src0, Src1, relu

    def mk():
        return Spec(
            body=relu(Src0 - Src1),
            reference=lambda in0, in1, s0, s1, imm2: np.maximum(
                in0.astype(np.float32) - in1.astype(np.float32), 0.0
            ),
        )

    return _register_op("SUB_RELU_ANT", mk)


def _build_nc():
    from contextlib import ExitStack
    from concourse import bacc, bass, tile, mybir

    MUL_SCAN2 = _register_mul_scan2()
    SUB_RELU = _register_sub_relu()
    AF = mybir.ActivationFunctionType
    f32 = mybir.dt.float32
    bf16 = mybir.dt.bfloat16
    f16 = mybir.dt.float16

    nc = bacc.Bacc("TRN2", target_bir_lowering=False, debug=False)

    opt_d = nc.declare_dram_parameter(
        "option", [B_LOCAL, OPT_RE_W], f16, isOutput=False
    )
    # inputs extended host-side with trailing [1.0, 0.0] to match the
    # [w | b | pad] segment layout
    x_d = nc.declare_dram_parameter("xext", [B_LOCAL, SEG1], f16, isOutput=False)
    wpack_d = nc.declare_dram_parameter("wpack", [P, WPACK_W], bf16, isOutput=False)
    bpack_d = nc.declare_dram_parameter("bpack", [P, BPACK_W], f32, isOutput=False)
    out_d = nc.declare_dram_parameter("out", [B_LOCAL, OUT_W], f32, isOutput=True)

    with tile.TileContext(nc) as tc, ExitStack() as ctx:
        wpool = ctx.enter_context(tc.tile_pool(name="weights", bufs=1))
        optp = ctx.enter_context(tc.tile_pool(name="opt", bufs=3))
        scanp = ctx.enter_context(tc.tile_pool(name="scan", bufs=2))
        xblk = ctx.enter_context(tc.tile_pool(name="xblk", bufs=2))
        hp = ctx.enter_context(tc.tile_pool(name="hyper", bufs=2))
        vp = ctx.enter_context(tc.tile_pool(name="vnet", bufs=2))
        outp = ctx.enter_context(tc.tile_pool(name="outst", bufs=4))
        psum = ctx.enter_context(
            tc.tile_pool(name="psum", bufs=6, space=bass.MemorySpace.PSUM)
        )
        psv = ctx.enter_context(
            tc.tile_pool(name="psv", bufs=2, space=bass.MemorySpace.PSUM)
        )

        out_tiles = [
            outp.tile([P, OUT_W], f32, tag="outst", name=f"out_st{g}")
            for g in range(NBLK)
        ]

        # ---- block-0 hypernet input DMAs first: they gate the DVE scan
        # pipeline, so they go ahead of the (PE-only) weight pack in the
        # sync-ring FIFO.
        L1W = 32 * SEG1  # 8256
        L1W4 = 16 * SEG1  # block-0 uses 4 half-size pieces for a faster start
        xb0 = xblk.tile([P, SEG1], f16, tag="xb")
        nc.sync.dma_start(xb0[:], x_d[0:P, :])
        B0_PIECES = [8, 8, 16, 16, 16]  # first pieces small for a fast start
        ot0 = []
        seg0 = 0
        for sc, nseg in enumerate(B0_PIECES):
            ot = optp.tile([P, L1W], f16, tag="opt", name=f"ot0_{sc}", bufs=5)
            nc.sync.dma_start(
                ot[:, 0 : nseg * SEG1],
                opt_d[0:P, seg0 * SEG1 : (seg0 + nseg) * SEG1],
            )
            ot0.append((ot, seg0, nseg))
            seg0 += nseg

        # ---- replicated matmul operands (single packed bf16 DMA) --------
        # (the dma_start itself is emitted inside the block-1 iteration so the
        # option stream keeps priority in the sync-ring FIFO)
        wp_sb = wpool.tile([P, WPACK_W], bf16)
        o = 0
        xt_sb = wp_sb[:, o : o + 2 * B_LOCAL].rearrange("p (k b) -> p k b", k=2)
        o += 2 * B_LOCAL
        w1_sb = wp_sb[:, o : o + 2 * H].rearrange("p (k h) -> p k h", k=2)
        o += 2 * H
        w2_sb = wp_sb[:, o : o + 8 * H].rearrange("p (k h) -> p k h", k=8)
        o += 8 * H
        w3_sb = wp_sb[:, o : o + 8]
        o += 8
        assert o == WPACK_W

        bp_sb = wpool.tile([P, BPACK_W], f32)
        nc.scalar.dma_start(bp_sb[:], bpack_d[:])
        b1_sb = bp_sb[:, 0:8]
        nb1_sb = bp_sb[:, 8:16]
        b2_sb = bp_sb[:, 16:24]
        nb2_sb = bp_sb[:, 24:32]
        b3r_sb = bp_sb[:, 32:33]

        # ---- hypernetwork (VectorE scans), per 128-sample block --------
        # option is host-repacked so segment o = [w_o | b_o | 0]; with the
        # activation vector extended by [1.0, 0.0], a prefix-sum diff over
        # one segment yields w_o . x + b_o directly.
        # scan tile cols: 0,1 = zero pad; element e of the stream at col e+2.
        def seg_ends(st, n_seg, seg):
            v = st[:, 3 : 3 + n_seg * seg].rearrange("p (o i) -> p o i", i=seg)
            return v[:, :, seg - 2 : seg - 1].squeeze(2)

        def seg_starts(st, n_seg, seg):
            v = st[:, 1 : 1 + n_seg * seg].rearrange("p (o i) -> p o i", i=seg)
            return v[:, :, 0:1].squeeze(2)

        for g in range(NBLK):
            rows = slice(g * P, (g + 1) * P)
            if g == 0:
                xb = xb0
            else:
                xb = xblk.tile([P, SEG1], f16, tag="xb")
                nc.sync.dma_start(xb[:], x_d[rows, :])

            # layer 1: 64 segments of 258
            h1 = hp.tile([P, SEG23], f16, tag="h1")
            if g == 0:
                pieces = ot0
            else:
                pieces = []
                for sc in range(2):
                    ot = optp.tile([P, L1W], f16, tag="opt", bufs=5)
                    nc.sync.dma_start(
                        ot[:], opt_d[rows, sc * L1W : (sc + 1) * L1W]
                    )
                    pieces.append((ot, sc * 32, 32))
            if g == 1:
                nc.sync.dma_start(wp_sb[:], wpack_d[:])
            for ot, seg0, nseg in pieces:
                w1l = nseg * SEG1
                st = scanp.tile([P, 3 + L1W], f16, tag="scan")
                nc.vector.memset(st[:, 0:2], 0.0)
                _emit_scan2(
                    nc,
                    MUL_SCAN2,
                    out=st[:, 2 : 2 + w1l],
                    in0=ot[:, 0:w1l],
                    in1=xb[:].unsqueeze(1).broadcast_to([P, nseg, SEG1]),
                )
                nc.vector._custom_dve(
                    SUB_RELU,
                    out=h1[:, seg0 : seg0 + nseg],
                    in0=seg_ends(st, nseg, SEG1),
                    in1=seg_starts(st, nseg, SEG1),
                )
            nc.vector.memset(h1[:, 64:65], 1.0)
            nc.vector.memset(h1[:, 65:66], 0.0)

            # layer 2: 64 segments of 66
            w2l = 64 * SEG23  # 4224
            ot2 = optp.tile([P, w2l], f16, tag="opt_s", bufs=3)
            nc.sync.dma_start(ot2[:], opt_d[rows, RE_L2 : RE_L2 + w2l])
            st2 = scanp.tile([P, 3 + w2l], f16, tag="scan_s", bufs=2)
            nc.vector.memset(st2[:, 0:2], 0.0)
            _emit_scan2(
                nc,
                MUL_SCAN2,
                out=st2[:, 2 : 2 + w2l],
                in0=ot2[:],
                in1=h1[:].unsqueeze(1).broadcast_to([P, 64, SEG23]),
            )
            h2 = hp.tile([P, SEG23], f16, tag="h2")
            nc.vector._custom_dve(
                SUB_RELU,
                out=h2[:, 0:64],
                in0=seg_ends(st2, 64, SEG23),
                in1=seg_starts(st2, 64, SEG23),
            )
            nc.vector.memset(h2[:, 64:65], 1.0)
            nc.vector.memset(h2[:, 65:66], 0.0)

            # layer 3: 32 segments of 66, no relu; diff lands in the output tile
            w3l = 32 * SEG23  # 2112
            ot3 = optp.tile([P, w2l], f16, tag="opt_s", bufs=3)
            nc.sync.dma_start(ot3[:, 0:w3l], opt_d[rows, RE_L3 : RE_L3 + w3l])
            st3 = scanp.tile([P, 3 + w2l], f16, tag="scan_s", bufs=2)
            nc.vector.memset(st3[:, 0:2], 0.0)
            _emit_scan2(
                nc,
                MUL_SCAN2,
                out=st3[:, 2 : 2 + w3l],
                in0=ot3[:, 0:w3l],
                in1=h2[:].unsqueeze(1).broadcast_to([P, 32, SEG23]),
            )
            nc.vector.tensor_sub(
                out_tiles[g][:, 1:33], seg_ends(st3, 32, SEG23), seg_starts(st3, 32, SEG23)
            )

        # ---- value network (TensorE bf16), all 512 samples at once -----
        # ELU+1 = relu(z) + exp(min(z,0)): two ACT passes per branch, then a
        # bf16 2x DVE add into the combined activation tile.
        h1_sb = vp.tile([P, 8, B_LOCAL], bf16, tag="h1v", bufs=1)
        for mt in range(8):
            ps = psum.tile([P, B_LOCAL], f32, tag="ps")
            for kt in range(2):
                nc.tensor.matmul(
                    ps[:],
                    w1_sb[:, kt, mt * P : (mt + 1) * P],
                    xt_sb[:, kt, :],
                    start=(kt == 0),
                    stop=(kt == 1),
                )
            r = vp.tile([P, B_LOCAL], bf16, tag="elu_r")
            nc.scalar.activation(r[:], ps[:], AF.Relu, bias=b1_sb[:, mt : mt + 1])
            u = vp.tile([P, B_LOCAL], f32, tag="elu_u")
            nc.scalar.activation(
                u[:], ps[:], AF.Relu, bias=nb1_sb[:, mt : mt + 1], scale=-1.0
            )
            e = vp.tile([P, B_LOCAL], bf16, tag="elu_e")
            nc.scalar.activation(e[:], u[:], AF.Exp, scale=-1.0)
            nc.vector.tensor_add(h1_sb[:, mt, :], r[:], e[:])

        h2_sb = vp.tile([P, 8, B_LOCAL], bf16, tag="h2v", bufs=1)
        for mt in range(8):
            ps = psum.tile([P, B_LOCAL], f32, tag="ps")
            for kt in range(8):
                nc.tensor.matmul(
                    ps[:],
                    w2_sb[:, kt, mt * P : (mt + 1) * P],
                    h1_sb[:, kt, :],
                    start=(kt == 0),
                    stop=(kt == 7),
                )
            r = vp.tile([P, B_LOCAL], bf16, tag="elu_r")
            nc.scalar.activation(r[:], ps[:], AF.Relu, bias=b2_sb[:, mt : mt + 1])
            u = vp.tile([P, B_LOCAL], f32, tag="elu_u")
            nc.scalar.activation(
                u[:], ps[:], AF.Relu, bias=nb2_sb[:, mt : mt + 1], scale=-1.0
            )
            e = vp.tile([P, B_LOCAL], bf16, tag="elu_e")
            nc.scalar.activation(e[:], u[:], AF.Exp, scale=-1.0)
            nc.vector.tensor_add(h2_sb[:, mt, :], r[:], e[:])

        for g in range(NBLK):
            pv = psv.tile([P, 1], f32, tag="pv")
            for kt in range(8):
                nc.tensor.matmul(
                    pv[:],
                    h2_sb[:, kt, g * P : (g + 1) * P],
                    w3_sb[:, kt : kt + 1],
                    start=(kt == 0),
                    stop=(kt == 7),
                )
            nc.scalar.activation(
                out_tiles[g][:, 0:1], pv[:], AF.Identity, bias=b3r_sb[:, 0:1]
            )

        for g in range(NBLK):
            rows = slice(g * P, (g + 1) * P)
            nc.scalar.dma_start(out_d[rows, :], out_tiles[g][:])

    nc.compile()
    return nc


def _get_nc():
    if "nc" not in _NC_CACHE:
        _NC_CACHE["nc"] = _build_nc()
    return _NC_CACHE["nc"]


def _prep_in_maps(inputs):
    x = np.ascontiguousarray(np.asarray(inputs["inputs"], dtype=np.float32))
    opt = np.asarray(inputs["option"], dtype=np.float32)
    w1 = np.asarray(inputs["w1"], dtype=np.float32)
    b1 = np.asarray(inputs["b1"], dtype=np.float32)
    w2 = np.asarray(inputs["w2"], dtype=np.float32)
    b2 = np.asarray(inputs["b2"], dtype=np.float32)
    w3 = np.asarray(inputs["w3"], dtype=np.float32)
    b3 = np.asarray(inputs["b3"], dtype=np.float32)

    opt_re = _repack_option(opt)
    x_ext = np.zeros((x.shape[0], SEG1), dtype=np.float16)
    x_ext[:, 0:Z] = x
    x_ext[:, Z] = 1.0

    # weight [K, M] with K across 128-partition tiles -> [128, n_k_tiles * M]
    def ktiled(a):  # a: [K, M]
        k, m = a.shape
        return a.reshape(k // P, P, m).transpose(1, 0, 2).reshape(P, -1)

    w1t = ktiled(w1.T)  # [128, 2*1024]
    w2t = ktiled(w2.T)  # [128, 8*1024]
    w3t = w3.reshape(8, P).T  # [128, 8]
    b1t = b1.reshape(8, P).T  # [128, 8]
    # device computes elu+1 (= relu(z)+exp(min(z,0))); fold the -1 into the
    # consumer's bias: b' = b - W.sum(axis=1)
    b2p = b2 - w2.sum(axis=1)
    b2t = b2p.reshape(8, P).T
    b3p = float(b3[0] - w3.sum())
    b3r = np.full((P, 1), b3p, dtype=np.float32)
    wtail = np.concatenate([w1t, w2t, w3t], axis=1)
    bpack = np.ascontiguousarray(
        np.concatenate([b1t, -b1t, b2t, -b2t, b3r], axis=1), dtype=np.float32
    )
    assert bpack.shape == (P, BPACK_W)

    in_maps = []
    for c in range(NCORES):
        sl = slice(c * B_LOCAL, (c + 1) * B_LOCAL)
        xs = np.ascontiguousarray(x[sl])
        xt = ktiled(xs.T)  # [128, 2*512]
        wpack = np.ascontiguousarray(_bf16(np.concatenate([xt, wtail], axis=1)))
        assert wpack.shape == (P, WPACK_W)
        in_maps.append(
            {
                "option": np.ascontiguousarray(opt_re[sl]),
                "xext": np.ascontiguousarray(x_ext[sl]),
                "wpack": wpack,
                "bpack": bpack,
            }
        )
    return in_maps


def _ensure_ntff_hook():
    """Provide antenv.axon_hooks (missing in this image) so trace=True works."""
    import sys
    import types

    if "antenv.axon_hooks" in sys.modules:
        return
    mod = types.ModuleType("antenv.axon_hooks")
    state = {"hook": None}
    mod.set_axon_ntff_profile_hook = lambda h: state.__setitem__("hook", h)
    mod.get_axon_ntff_profile_hook = lambda: state["hook"]
    sys.modules["antenv.axon_hooks"] = mod
    import antenv

    antenv.axon_hooks = mod
    try:
        from trn_agent_boot.trn_boot import _ntff_profile_via_ctypes

        hook = _ntff_profile_via_ctypes("/opt/axon/libaxon_pjrt.so")
        mod.set_axon_ntff_profile_hook(hook)
    except Exception as e:  # degrade: tracing skipped, run still works
        print(f"ntff hook setup failed: {e}")


def run(inputs, trace=False):
    """Returns (full_output [4096, 33] float32, exec_time_ns or None)."""
    from concourse.bass_utils import run_bass_kernel_spmd

    if trace:
        _ensure_ntff_hook()
    nc = _get_nc()
    in_maps = _prep_in_maps(inputs)
    res = run_bass_kernel_spmd(nc, in_maps, core_ids=list(range(NCORES)), trace=trace)
    out = np.concatenate([res.results[i]["out"] for i in range(NCORES)], axis=0)
    return out.astype(np.float32), res.exec_time_ns


def kernel(**inputs):
    out, _ = run(inputs, trace=False)
    return out


# revision 50
# speedup vs baseline: 1.0475x; 1.0475x over previous
"""Trainium2 Bass kernel for nn_ActorCritic (value MLP + per-sample hypernetwork).

Sharding: pure data parallel. Batch 4096 split as 512 samples per core across
8 NeuronCores; the small value-network weights are replicated (host
pre-transposed/packed so the device never transposes anything).

Per-core work:
  - Value net (TensorE, bf16): X^T [256,512] -> W1^T matmuls -> ELU -> W2^T
    -> ELU -> w3 -> value [512]. Kept in [feature, batch] layout so biases are
    per-partition ACT scalars. ELU(z) = relu(z) + exp(min(z,0)) - 1; the -1
    is folded into the *next* layer's bias on the host (b' = b - W.sum(axis=1)),
    and the relu/exp branches are kept as separate bf16 tiles -- the next
    layer's matmul runs over both and accumulates in PSUM (linearity), so no
    elementwise add is ever needed.
  - Hypernetwork (VectorE): option rows hold per-sample MLP weights
    (64x256, 64, 64x64, 64, 32x64, 32 packed), stored bf16. With samples on
    partitions, a custom DVE op computes a running dot product
    scan(ADD, Src0*Src1) over the weight stream against a stride-0-broadcast
    activation vector; segment sums are extracted by strided subtraction of
    the prefix sums (GpSimd does the small fix-up ops).
"""

import numpy as np

P = 128
B_LOCAL = 512
NBLK = B_LOCAL // P  # 4 blocks of 128 samples
NCORES = 8
Z = 256
H = 1024
OPT_W = 22688
OUT_W = 33

# hypernet packing offsets (HyperOption z=256->64->64->32)
OFF_W1, OFF_B1 = 0, 16384
OFF_W2, OFF_B2 = 16448, 20544
OFF_W3, OFF_B3 = 20608, 22656

# device-side repacked option stream: per layer, segment o = [w_o | b_o | 0],
# so a prefix-sum difference over one segment directly yields w_o . x + b_o.
# Segments are padded to even length so the DVE 2x packed-pair mode applies.
# L1: 64 segments of 258; L2: 64 of 66; L3: 32 of 66.
SEG1, SEG23 = 258, 66
RE_L2 = 64 * SEG1  # 16512
RE_L3 = RE_L2 + 64 * SEG23  # 20736
OPT_RE_W = RE_L3 + 32 * SEG23  # 22848


def _repack_option(opt):
    """[B, 22688] f32 -> [B, 22848] fp16 padded-segment layout."""
    B = opt.shape[0]
    out = np.zeros((B, OPT_RE_W), dtype=np.float16)
    l1 = out[:, :RE_L2].reshape(B, 64, SEG1)
    l1[:, :, 0:256] = opt[:, OFF_W1 : OFF_W1 + 16384].reshape(B, 64, 256)
    l1[:, :, 256] = opt[:, OFF_B1 : OFF_B1 + 64]
    l2 = out[:, RE_L2:RE_L3].reshape(B, 64, SEG23)
    l2[:, :, 0:64] = opt[:, OFF_W2 : OFF_W2 + 4096].reshape(B, 64, 64)
    l2[:, :, 64] = opt[:, OFF_B2 : OFF_B2 + 64]
    l3 = out[:, RE_L3:].reshape(B, 32, SEG23)
    l3[:, :, 0:64] = opt[:, OFF_W3 : OFF_W3 + 2048].reshape(B, 32, 64)
    l3[:, :, 64] = opt[:, OFF_B3 : OFF_B3 + 32]
    return out

# packed bf16 value-net matmul operands: xt | w1t | w2t | w3t
WPACK_W = 2 * 512 + 2 * 1024 + 8 * 1024 + 8  # 11272
# packed f32 biases: b1 | -b1 | b2' | -b2' | b3'
BPACK_W = 8 + 8 + 8 + 8 + 1  # 33

_NC_CACHE = {}
USE_2X = True  # set False to force the REGULAR (1x) scan program


def _bf16(a):
    import ml_dtypes

    return np.asarray(a, dtype=ml_dtypes.bfloat16)


def _register_op(name, make_spec):
    """Register (once) a custom DVE op by name; returns the DveOp."""
    from concourse.dve_spec import lower
    from concourse import dve_ops
    from concourse.dve_uop import DveOpSpec

    for op in dve_ops.OPS:
        if op.name == name:
            return op
    spec = make_spec()
    row = dve_ops._CUSTOM_DVE_ROW_BASE + len(dve_ops.OPS)
    assert row < 0x20
    dve_ops._SUB_OPCODE_FOR_NAME[name] = row
    shas = {}
    for ver in ("v3", "v4"):
        tmp = DveOpSpec(name=name, opcode=row, uops=lower(spec, ver=ver), rd1_en=True)
        shas[ver] = tmp.sha(ver)
    op = dve_ops.DveOp(name, spec, subdim=False, uops_sha=shas)
    dve_ops.OPS.append(op)
    dve_ops.CUSTOM_DVE_SPECS[name] = spec
    return op


def _register_mul_scan():
    """out[p,k] = sum_{j<=k} in0[p,j]*in1[p,j] (fp32 running dot product)."""
    from concourse.dve_spec import Spec, Src0, Src1, AluOp, scan

    def mk():
        def _ref(in0, in1, s0, s1, imm2):
            p = in0.shape[0]
            prod = in0.astype(np.float32).reshape(p, -1) * in1.astype(
                np.float32
            ).reshape(p, -1)
            return np.cumsum(prod, axis=-1).reshape(in0.shape).astype(np.float32)

        return Spec(body=scan(AluOp.ADD, Src0 * Src1), reference=_ref)

    return _register_op("MUL_SCAN_ANT", mk)


def _register_mul_scan2():
    """Running dot product with a hand-authored 2X_1PORT uop variant.

    REGULAR program comes from lower(); the 2x program processes two packed
    fp16 elements per cycle: m0=a0*b0, m1=a1*b1, q=m0+m1, s=q+s_prev
    (single-block feedback), lo=s-m1; writes (lo, s) to WR0_LO/WR0_HI.
    The engine falls back to REGULAR at runtime if the access pattern is
    ineligible, so correctness never depends on 2x engaging.
    """
    from concourse.dve_spec import Spec, Src0, Src1, AluOp as SAluOp, scan, lower
    from concourse import dve_ops
    from concourse.dve_uop import (
        DveOpSpec,
        UopConfig,
        AluOp as UAluOp,
        AluInp,
        DelayInp,
        InpSel,
        OutSel,
        OutPath,
        Trigger,
        ENABLE,
    )

    name = "MUL_SCAN2_ANT"
    for op in dve_ops.OPS:
        if op.name == name:
            return op

    def _ref(in0, in1, s0, s1, imm2):
        p = in0.shape[0]
        prod = in0.astype(np.float32).reshape(p, -1) * in1.astype(
            np.float32
        ).reshape(p, -1)
        return np.cumsum(prod, axis=-1).reshape(in0.shape).astype(np.float32)

    spec = Spec(body=scan(SAluOp.ADD, Src0 * Src1), reference=_ref)

    def mk_seed():
        u = UopConfig()
        u.enable_input(InpSel.ZERO, 5)  # delay chain 4 carries 0 to block 3
        b = u.datapath_config
        for k in (0, 1, 2):
            b[k].pass_through_alu()
            b[k].pass_through_delay(4)
        b[3].enable_alu(UAluOp.BYPASS, AluInp.PREV_DELAY_4, AluInp.PREV_DELAY_4)
        for k in (4, 5, 6, 7):
            b[k].pass_through_alu()
        u.repeat_count = 1
        u.trigger = (Trigger.COUNT, Trigger.NONE, Trigger.NONE)
        u.next_uop = (1, 0, 0)
        return u

    def mk_steady():
        u = UopConfig()
        u.enable_input(InpSel.SRC_0, 1)
        u.enable_input(InpSel.SRC_1, 2)
        u.enable_input(InpSel.SRC_0_HI, 3)
        u.enable_input(InpSel.SRC_1_HI, 4)
        b = u.datapath_config
        # m0 = a0*b0; carry (a1, b1) forward
        b[0].enable_alu(UAluOp.MULTIPLY, AluInp.PREV_DELAY_0, AluInp.PREV_DELAY_1)
        b[0].pass_through_delay(2, 3)
        # m1 = a1*b1; chain0 <- m0
        b[1].enable_alu(UAluOp.MULTIPLY, AluInp.PREV_DELAY_2, AluInp.PREV_DELAY_3)
        b[1].enable_delay_from_src(DelayInp.PREV_ALU_OUT, 0)
        # q = m1 + m0; chain1 <- m1
        b[2].enable_alu(UAluOp.ADD, AluInp.PREV_ALU_OUT, AluInp.PREV_DELAY_0)
        b[2].enable_delay_from_src(DelayInp.PREV_ALU_OUT, 1)
        # s = q + s_prev (feedback); carry m1
        b[3].enable_alu(UAluOp.ADD, AluInp.PREV_ALU_OUT, AluInp.CURR_ALU_OUT)
        b[3].pass_through_delay(1)
        # lo = s - m1; chain0 <- s
        b[4].enable_alu(UAluOp.SUBTRACT, AluInp.PREV_ALU_OUT, AluInp.PREV_DELAY_1)
        b[4].enable_delay_from_src(DelayInp.PREV_ALU_OUT, 0)
        for k in (5, 6, 7):
            b[k].pass_through_alu()
            b[k].pass_through_delay(0)
        u.enable_output(OutSel.ALU_OUT, OutPath.WR0_LO)
        u.enable_output(OutSel.DELAY_0, OutPath.WR0_HI)
        u.require_inp0 = ENABLE
        u.require_inp1 = ENABLE
        u.trigger = (Trigger.SRC_TENSOR_DONE, Trigger.NONE, Trigger.NONE)
        u.next_uop = (0, 0, 0)
        return u

    row = dve_ops._CUSTOM_DVE_ROW_BASE + len(dve_ops.OPS)
    assert row < 0x20
    dve_ops._SUB_OPCODE_FOR_NAME[name] = row

    class Scan2Op:
        def __init__(self):
            self.name = name
            self.spec = spec
            self.subdim = False
            self._cache = {}

        def compile(self, ver):
            if ver not in self._cache:
                kw = {}
                if ver == "v3":
                    kw = dict(uops_2x=[mk_seed(), mk_steady()], perf_max=1)
                self._cache[ver] = DveOpSpec(
                    name=name,
                    opcode=row,
                    uops=lower(spec, ver=ver),
                    rd1_en=True,
                    **kw,
                )
            return self._cache[ver]

    op = Scan2Op()
    dve_ops.OPS.append(op)
    dve_ops.CUSTOM_DVE_SPECS[name] = spec
    return op


def _emit_scan2(nc, op, out, in0, in1):
    """nc.vector._custom_dve equivalent with the 2x perf-mode bit set."""
    from concourse import bass_isa, mybir

    v = nc.vector
    from concourse.dve_table_gen import dve_ver_for

    op.compile(dve_ver_for(nc.trn_type))
    if op.name not in nc.m.ant_custom_dve_ops:
        nc.m.ant_custom_dve_ops = sorted({*nc.m.ant_custom_dve_ops, op.name})
    from concourse.dve_ops import get_dve_sub_opcode

    shape = (
        bass_isa.CustomDveShape.STT
        if len(in1.shape) > 2
        else bass_isa.CustomDveShape.TTSS
    )
    isa_opcode = nc.isa.Opcode[
        f"NEURON_ISA_TPB_OPCODE_CUSTOM_DVE_ANT_{shape.slot()}"
    ].value
    zero = mybir.ImmediateValue(dtype=mybir.dt.float32, value=0.0)
    return v.add_instruction(
        bass_isa.InstCustomDveAnt(
            name=nc.get_next_instruction_name(),
            op_name=op.name,
            rd1_en=True,
            subdim=0,
            imm2=0.0,
            shape=shape,
            row=get_dve_sub_opcode(op.name),
            isa_opcode=isa_opcode,
            ins=[
                v.lower_ap(in0, for_isa=True, opt=True),
                v.lower_ap(in1, for_isa=True, opt=True),
                zero,
                zero,
            ],
            outs=[v.lower_ap(out, for_isa=True, opt=True)],
            perf_max=1 if USE_2X else 0,
        )
    )


def _register_sub_relu():
    """out = relu(in0 - in1)."""
    from concourse.dve_spec import Spec, Src0, Src1, relu

    def mk():
        return Spec(
            body=relu(Src0 - Src1),
            reference=lambda in0, in1, s0, s1, imm2: np.maximum(
                in0.astype(np.float32) - in1.astype(np.float32), 0.0
            ),
        )

    return _register_op("SUB_RELU_ANT", mk)


def _build_nc():
    from contextlib import ExitStack
    from concourse import bacc, bass, tile, mybir

    MUL_SCAN2 = _register_mul_scan2()
    SUB_RELU = _register_sub_relu()
    AF = mybir.ActivationFunctionType
    f32 = mybir.dt.float32
    bf16 = mybir.dt.bfloat16
    f16 = mybir.dt.float16

    nc = bacc.Bacc("TRN2", target_bir_lowering=False, debug=False)

    opt_d = nc.declare_dram_parameter(
        "option", [B_LOCAL, OPT_RE_W], f16, isOutput=False
    )
    # inputs extended host-side with trailing [1.0, 0.0] to match the
    # [w | b | pad] segment layout
    x_d = nc.declare_dram_parameter("xext", [B_LOCAL, SEG1], f16, isOutput=False)
    wpack_d = nc.declare_dram_parameter("wpack", [P, WPACK_W], bf16, isOutput=False)
    bpack_d = nc.declare_dram_parameter("bpack", [P, BPACK_W], f32, isOutput=False)
    out_d = nc.declare_dram_parameter("out", [B_LOCAL, OUT_W], f32, isOutput=True)

    with tile.TileContext(nc) as tc, ExitStack() as ctx:
        wpool = ctx.enter_context(tc.tile_pool(name="weights", bufs=1))
        optp = ctx.enter_context(tc.tile_pool(name="opt", bufs=3))
        scanp = ctx.enter_context(tc.tile_pool(name="scan", bufs=2))
        xblk = ctx.enter_context(tc.tile_pool(name="xblk", bufs=2))
        hp = ctx.enter_context(tc.tile_pool(name="hyper", bufs=2))
        vp = ctx.enter_context(tc.tile_pool(name="vnet", bufs=2))
        outp = ctx.enter_context(tc.tile_pool(name="outst", bufs=4))
        psum = ctx.enter_context(
            tc.tile_pool(name="psum", bufs=6, space=bass.MemorySpace.PSUM)
        )
        psv = ctx.enter_context(
            tc.tile_pool(name="psv", bufs=2, space=bass.MemorySpace.PSUM)
        )

        out_tiles = [
            outp.tile([P, OUT_W], f32, tag="outst", name=f"out_st{g}")
            for g in range(NBLK)
        ]

        # ---- block-0 hypernet input DMAs first: they gate the DVE scan
        # pipeline, so they go ahead of the (PE-only) weight pack in the
        # sync-ring FIFO.
        L1W = 32 * SEG1  # 8256
        L1W4 = 16 * SEG1  # block-0 uses 4 half-size pieces for a faster start
        xb0 = xblk.tile([P, SEG1], f16, tag="xb")
        nc.sync.dma_start(xb0[:], x_d[0:P, :])
        B0_PIECES = [8, 8, 16, 16, 16]  # first pieces small for a fast start
        ot0 = []
        seg0 = 0
        for sc, nseg in enumerate(B0_PIECES):
            ot = optp.tile([P, L1W], f16, tag="opt", name=f"ot0_{sc}", bufs=5)
            nc.sync.dma_start(
                ot[:, 0 : nseg * SEG1],
                opt_d[0:P, seg0 * SEG1 : (seg0 + nseg) * SEG1],
            )
            ot0.append((ot, seg0, nseg))
            seg0 += nseg

        # ---- replicated matmul operands (packed bf16) -------------------
        # xt+w1 (needed first by PE layer 1) DMA'd early; the w2 bulk is
        # emitted inside block-1 so the option stream keeps FIFO priority.
        WP_SPLIT = 2 * B_LOCAL + 2 * H  # 3072
        wp_sb = wpool.tile([P, WPACK_W], bf16)
        nc.sync.dma_start(wp_sb[:, 0:WP_SPLIT], wpack_d[:, 0:WP_SPLIT])
        o = 0
        xt_sb = wp_sb[:, o : o + 2 * B_LOCAL].rearrange("p (k b) -> p k b", k=2)
        o += 2 * B_LOCAL
        w1_sb = wp_sb[:, o : o + 2 * H].rearrange("p (k h) -> p k h", k=2)
        o += 2 * H
        w2_sb = wp_sb[:, o : o + 8 * H].rearrange("p (k h) -> p k h", k=8)
        o += 8 * H
        w3_sb = wp_sb[:, o : o + 8]
        o += 8
        assert o == WPACK_W

        bp_sb = wpool.tile([P, BPACK_W], f32)
        nc.scalar.dma_start(bp_sb[:], bpack_d[:])
        b1_sb = bp_sb[:, 0:8]
        nb1_sb = bp_sb[:, 8:16]
        b2_sb = bp_sb[:, 16:24]
        nb2_sb = bp_sb[:, 24:32]
        b3r_sb = bp_sb[:, 32:33]

        # ---- hypernetwork (VectorE scans), per 128-sample block --------
        # option is host-repacked so segment o = [w_o | b_o | 0]; with the
        # activation vector extended by [1.0, 0.0], a prefix-sum diff over
        # one segment yields w_o . x + b_o directly.
        # scan tile cols: 0,1 = zero pad; element e of the stream at col e+2.
        def seg_ends(st, n_seg, seg):
            v = st[:, 3 : 3 + n_seg * seg].rearrange("p (o i) -> p o i", i=seg)
            return v[:, :, seg - 2 : seg - 1].squeeze(2)

        def seg_starts(st, n_seg, seg):
            v = st[:, 1 : 1 + n_seg * seg].rearrange("p (o i) -> p o i", i=seg)
            return v[:, :, 0:1].squeeze(2)

        for g in range(NBLK):
            rows = slice(g * P, (g + 1) * P)
            if g == 0:
                xb = xb0
            else:
                xb = xblk.tile([P, SEG1], f16, tag="xb")
                nc.sync.dma_start(xb[:], x_d[rows, :])

            # layer 1: 64 segments of 258
            h1 = hp.tile([P, SEG23], f16, tag="h1")
            if g == 0:
                pieces = ot0
            else:
                pieces = []
                for sc in range(2):
                    ot = optp.tile([P, L1W], f16, tag="opt", bufs=5)
                    nc.sync.dma_start(
                        ot[:], opt_d[rows, sc * L1W : (sc + 1) * L1W]
                    )
                    pieces.append((ot, sc * 32, 32))
            if g == 1:
                nc.sync.dma_start(
                    wp_sb[:, WP_SPLIT:WPACK_W], wpack_d[:, WP_SPLIT:WPACK_W]
                )
            for ot, seg0, nseg in pieces:
                w1l = nseg * SEG1
                st = scanp.tile([P, 3 + L1W], f16, tag="scan")
                nc.vector.memset(st[:, 0:2], 0.0)
                _emit_scan2(
                    nc,
                    MUL_SCAN2,
                    out=st[:, 2 : 2 + w1l],
                    in0=ot[:, 0:w1l],
                    in1=xb[:].unsqueeze(1).broadcast_to([P, nseg, SEG1]),
                )
                nc.vector._custom_dve(
                    SUB_RELU,
                    out=h1[:, seg0 : seg0 + nseg],
                    in0=seg_ends(st, nseg, SEG1),
                    in1=seg_starts(st, nseg, SEG1),
                )
            nc.vector.memset(h1[:, 64:65], 1.0)
            nc.vector.memset(h1[:, 65:66], 0.0)

            # layer 2: 64 segments of 66
            w2l = 64 * SEG23  # 4224
            ot2 = optp.tile([P, w2l], f16, tag="opt_s", bufs=3)
            nc.sync.dma_start(ot2[:], opt_d[rows, RE_L2 : RE_L2 + w2l])
            st2 = scanp.tile([P, 3 + w2l], f16, tag="scan_s", bufs=2)
            nc.vector.memset(st2[:, 0:2], 0.0)
            _emit_scan2(
                nc,
                MUL_SCAN2,
                out=st2[:, 2 : 2 + w2l],
                in0=ot2[:],
                in1=h1[:].unsqueeze(1).broadcast_to([P, 64, SEG23]),
            )
            h2 = hp.tile([P, SEG23], f16, tag="h2")
            nc.vector._custom_dve(
                SUB_RELU,
                out=h2[:, 0:64],
                in0=seg_ends(st2, 64, SEG23),
                in1=seg_starts(st2, 64, SEG23),
            )
            nc.vector.memset(h2[:, 64:65], 1.0)
            nc.vector.memset(h2[:, 65:66], 0.0)

            # layer 3: 32 segments of 66, no relu; diff lands in the output tile
            w3l = 32 * SEG23  # 2112
            ot3 = optp.tile([P, w2l], f16, tag="opt_s", bufs=3)
            nc.sync.dma_start(ot3[:, 0:w3l], opt_d[rows, RE_L3 : RE_L3 + w3l])
            st3 = scanp.tile([P, 3 + w2l], f16, tag="scan_s", bufs=2)
            nc.vector.memset(st3[:, 0:2], 0.0)
            _emit_scan2(
                nc,
                MUL_SCAN2,
                out=st3[:, 2 : 2 + w3l],
                in0=ot3[:, 0:w3l],
                in1=h2[:].unsqueeze(1).broadcast_to([P, 32, SEG23]),
            )
            nc.vector.tensor_sub(
                out_tiles[g][:, 1:33], seg_ends(st3, 32, SEG23), seg_starts(st3, 32, SEG23)
            )

        # ---- value network (TensorE bf16), all 512 samples at once -----
        # ELU+1 = relu(z) + exp(min(z,0)): two ACT passes per branch, then a
        # bf16 2x DVE add into the combined activation tile.
        h1_sb = vp.tile([P, 8, B_LOCAL], bf16, tag="h1v", bufs=1)
        for mt in range(8):
            ps = psum.tile([P, B_LOCAL], f32, tag="ps")
            for kt in range(2):
                nc.tensor.matmul(
                    ps[:],
                    w1_sb[:, kt, mt * P : (mt + 1) * P],
                    xt_sb[:, kt, :],
                    start=(kt == 0),
                    stop=(kt == 1),
                )
            r = vp.tile([P, B_LOCAL], bf16, tag="elu_r")
            nc.scalar.activation(r[:], ps[:], AF.Relu, bias=b1_sb[:, mt : mt + 1])
            u = vp.tile([P, B_LOCAL], f32, tag="elu_u")
            nc.scalar.activation(
                u[:], ps[:], AF.Relu, bias=nb1_sb[:, mt : mt + 1], scale=-1.0
            )
            e = vp.tile([P, B_LOCAL], bf16, tag="elu_e")
            nc.scalar.activation(e[:], u[:], AF.Exp, scale=-1.0)
            nc.vector.tensor_add(h1_sb[:, mt, :], r[:], e[:])

        h2_sb = vp.tile([P, 8, B_LOCAL], bf16, tag="h2v", bufs=1)
        for mt in range(8):
            ps = psum.tile([P, B_LOCAL], f32, tag="ps")
            for kt in range(8):
                nc.tensor.matmul(
                    ps[:],
                    w2_sb[:, kt, mt * P : (mt + 1) * P],
                    h1_sb[:, kt, :],
                    start=(kt == 0),
                    stop=(kt == 7),
                )
            r = vp.tile([P, B_LOCAL], bf16, tag="elu_r")
            nc.scalar.activation(r[:], ps[:], AF.Relu, bias=b2_sb[:, mt : mt + 1])
            u = vp.tile([P, B_LOCAL], f32, tag="elu_u")
            nc.scalar.activation(
                u[:], ps[:], AF.Relu, bias=nb2_sb[:, mt : mt + 1], scale=-1.0
            )
            e = vp.tile([P, B_LOCAL], bf16, tag="elu_e")
            nc.scalar.activation(e[:], u[:], AF.Exp, scale=-1.0)
            nc.vector.tensor_add(h2_sb[:, mt, :], r[:], e[:])

        for g in range(NBLK):
            pv = psv.tile([P, 1], f32, tag="pv")
            for kt in range(8):
                nc.tensor.matmul(
                    pv[:],
                    h2_sb[:, kt, g * P : (g + 1) * P],
                    w3_sb[:, kt : kt + 1],
                    start=(kt == 0),
                    stop=(kt == 7),
                )
            nc.scalar.activation(
                out_tiles[g][:, 0:1], pv[:], AF.Identity, bias=b3r_sb[:, 0:1]
            )

        for g in range(NBLK):
            rows = slice(g * P, (g + 1) * P)
            nc.scalar.dma_start(out_d[rows, :], out_tiles[g][:])

    nc.compile()
    return nc


def _get_nc():
    if "nc" not in _NC_CACHE:
        _NC_CACHE["nc"] = _build_nc()
    return _NC_CACHE["nc"]


def _prep_in_maps(inputs):
    x = np.ascontiguousarray(np.asarray(inputs["inputs"], dtype=np.float32))
    opt = np.asarray(inputs["option"], dtype=np.float32)
    w1 = np.asarray(inputs["w1"], dtype=np.float32)
    b1 = np.asarray(inputs["b1"], dtype=np.float32)
    w2 = np.asarray(inputs["w2"], dtype=np.float32)
    b2 = np.asarray(inputs["b2"], dtype=np.float32)
    w3 = np.asarray(inputs["w3"], dtype=np.float32)
    b3 = np.asarray(inputs["b3"], dtype=np.float32)

    opt_re = _repack_option(opt)
    x_ext = np.zeros((x.shape[0], SEG1), dtype=np.float16)
    x_ext[:, 0:Z] = x
    x_ext[:, Z] = 1.0

    # weight [K, M] with K across 128-partition tiles -> [128, n_k_tiles * M]
    def ktiled(a):  # a: [K, M]
        k, m = a.shape
        return a.reshape(k // P, P, m).transpose(1, 0, 2).reshape(P, -1)

    w1t = ktiled(w1.T)  # [128, 2*1024]
    w2t = ktiled(w2.T)  # [128, 8*1024]
    w3t = w3.reshape(8, P).T  # [128, 8]
    b1t = b1.reshape(8, P).T  # [128, 8]
    # device computes elu+1 (= relu(z)+exp(min(z,0))); fold the -1 into the
    # consumer's bias: b' = b - W.sum(axis=1)
    b2p = b2 - w2.sum(axis=1)
    b2t = b2p.reshape(8, P).T
    b3p = float(b3[0] - w3.sum())
    b3r = np.full((P, 1), b3p, dtype=np.float32)
    wtail = np.concatenate([w1t, w2t, w3t], axis=1)
    bpack = np.ascontiguousarray(
        np.concatenate([b1t, -b1t, b2t, -b2t, b3r], axis=1), dtype=np.float32
    )
    assert bpack.shape == (P, BPACK_W)

    in_maps = []
    for c in range(NCORES):
        sl = slice(c * B_LOCAL, (c + 1) * B_LOCAL)
        xs = np.ascontiguousarray(x[sl])
        xt = ktiled(xs.T)  # [128, 2*512]
        wpack = np.ascontiguousarray(_bf16(np.concatenate([xt, wtail], axis=1)))
        assert wpack.shape == (P, WPACK_W)
        in_maps.append(
            {
                "option": np.ascontiguousarray(opt_re[sl]),
                "xext": np.ascontiguousarray(x_ext[sl]),
                "wpack": wpack,
                "bpack": bpack,
            }
        )
    return in_maps


def _ensure_ntff_hook():
    """Provide antenv.axon_hooks (missing in this image) so trace=True works."""
    import sys
    import types

    if "antenv.axon_hooks" in sys.modules:
        return
    mod = types.ModuleType("antenv.axon_hooks")
    state = {"hook": None}
    mod.set_axon_ntff_profile_hook = lambda h: state.__setitem__("hook", h)
    mod.get_axon_ntff_profile_hook = lambda: state["hook"]
    sys.modules["antenv.axon_hooks"] = mod
    import antenv

    antenv.axon_hooks = mod
    try:
        from trn_agent_boot.trn_boot import _ntff_profile_via_ctypes

        hook = _ntff_profile_via_ctypes("/opt/axon/libaxon_pjrt.so")
        mod.set_axon_ntff_profile_hook(hook)
    except Exception as e:  # degrade: tracing skipped, run still works
        print(f"ntff hook setup failed: {e}")


def run(inputs, trace=False):
    """Returns (full_output [4096, 33] float32, exec_time_ns or None)."""
    from concourse.bass_utils import run_bass_kernel_spmd

    if trace:
        _ensure_ntff_hook()
    nc = _get_nc()
    in_maps = _prep_in_maps(inputs)
    res = run_bass_kernel_spmd(nc, in_maps, core_ids=list(range(NCORES)), trace=trace)
    out = np.concatenate([res.results[i]["out"] for i in range(NCORES)], axis=0)
    return out.astype(np.float32), res.exec_time_ns


def kernel(**inputs):
    out, _ = run(inputs, trace=False)
    return out
